# revision 37
# baseline (speedup 1.0000x reference)
"""Trainium2 Bass kernel: single-head causal attention (v3.3).

Problem: x[4,2048,1024] f32; q/k/v = x@W* + b* (head dim 128);
out = softmax(causal(q k^T / sqrt(128))) @ v.

Sharding: 8 cores = 4 batches x 2 causal "wedges". Within a batch, the 16
query blocks (128 rows each) are interleaved between the two cores
(h=0 takes odd global blocks, h=1 even) so both cores carry an identical
static schedule: slot p attends exactly L_p = 2p+2 local key blocks.
Per-core key order is a host-side permutation (h=0 identity, h=1
adjacent-pair swap) that puts slot p's own (diagonal) block at local
position 2p+1; the wedge difference is carried by a mask input, so a
single NEFF serves all 8 cores (SPMD).

v3.3 design notes (trace-driven):
  - ALL THREE projections (q, k, v) run in fp8 e4m3 DoubleRow from ONE
    2MB x8 stream; the 4MB bf16 x stream of earlier versions is gone.
    fp8 v is accurate enough because out is a softmax-weighted average:
    v-hat quantization errors are ~iid across keys and shrink by
    sqrt(n_eff) under averaging. Total input = 3.26MB, on chip by ~15us.
  - DMA engines round-robin FAIRLY across in-flight transfers (share ~
    transfer count, issue order only staggers); x8 is split into 8
    sub-transfers so it dominates the wire from the start.
  - x8 is shipped KEY-CHUNK-major (4 chunks x 512 keys, each with all
    1024 contraction rows) so q/k projections complete per-chunk and the
    S^T/exp chain (ACT is the serial mid-phase resource) starts ~14us.
    Chunks 0,1 are the own-query columns (feed qT and kT positions 0..7),
    chunks 2,3 feed kT positions 8..15.
  - kT is stored as four independent [128,512] tiles (kt0..kt3) and qT as
    two, each written by exactly one psum copy (dep tracking is
    whole-tile). ACT copies only kt0; DVE does the rest.
  - PE pstate ramps to full clock only after ~3us of CONTINUOUS work:
    warmup matmuls bridge until x8 chunk0 lands, no gaps.
  - S^T per key block j into a 2-bank [128,1024] psum tile (pieces split
    at the bank boundary), ONE exp per block; 16-exp chain on ACT.
  - v per key block in [key, dk] orientation via DR (lhsT = fp8 x8 key
    block [128,2,128], rhs = Wv8 chunk): 4 matmuls x 128 cols per block.
    bv folds into the output epilogue (softmax weights sum to 1):
    out = o_ps*rcp + bv via one scalar_tensor_tensor.
  - PV bursts (denominator via the v_aug ones-column) chase the exp
    chain; all v blocks are projectable by ~14us so there is no late-v
    tail.
  - PSUM: kq per-chunk [128,512] tiles (kps x2 + qps x2, 4 banks)
    concurrent with spool 2x[128,1024] (4 banks); opool(3)+vpool(1)
    reuse the kq banks after release.
"""

import numpy as np

B, T, D, DK = 4, 2048, 1024, 128
NBLK = T // 128      # 16 key blocks per core
NSLOT = 8            # q slots per core (NSLOT*128 = 1024 q rows)
NCHUNK = D // 128    # bf16 contraction chunks (hi-precision v blocks)
NDC = D // 256       # fp8 double-chunks (all projections)
NKC = 4              # x8 key chunks (512 keys each)
KCW = T // NKC
SCALE = 1.0 / np.sqrt(np.float32(DK))
WS = 8.0             # fp8 weight prescale (power of 2; undone in psum copy)
WARMUP_MMS = 15

_built = None


def _build():
    from contextlib import ExitStack

    import concourse.mybir as mybir
    import concourse.tile as tile
    from concourse import bacc

    f32 = mybir.dt.float32
    bf16 = mybir.dt.bfloat16
    fp8 = mybir.dt.float8e4
    Act = mybir.ActivationFunctionType
    Alu = mybir.AluOpType
    DR = mybir.MatmulPerfMode.DoubleRow

    nc = bacc.Bacc("TRN2", target_bir_lowering=False, debug=False, num_devices=8)

    # all bulk inputs partition-major with large contiguous runs (DMA packet
    # size = run size; <4KB runs halve wire throughput)
    x8p = nc.dram_tensor("x8p", [128, NKC * NDC * 2 * KCW], fp8,
                         kind="ExternalInput").ap()
    cstf8 = nc.dram_tensor("cstf8", [128, 3 * NDC * 2 * DK], fp8,
                           kind="ExternalInput").ap()
    cst32 = nc.dram_tensor("cst32", [128, 3 + DK], f32, kind="ExternalInput").ap()
    cst16 = nc.dram_tensor("cst16", [128, NCHUNK * DK + 256], bf16,
                           kind="ExternalInput").ap()
    xhip = nc.dram_tensor("xhip", [128, NCHUNK * 256], bf16,
                          kind="ExternalInput").ap()
    o = nc.dram_tensor("o", [NSLOT * 128, DK], f32, kind="ExternalOutput").ap()

    with tile.TileContext(nc) as tc, ExitStack() as ctx:
        const = ctx.enter_context(tc.tile_pool(name="const", bufs=1))
        sbufs = ctx.enter_context(tc.tile_pool(name="sbufs", bufs=1))
        x8_pool = ctx.enter_context(tc.tile_pool(name="x8_pool", bufs=2))
        out_pool = ctx.enter_context(tc.tile_pool(name="out_pool", bufs=3))

        # ---- input DMAs. x8 key chunks are PAIRED into two 1MB transfers
        # with 8KB runs. Wave 1 (weights, biases, chunks 0,1) free-runs;
        # wave 2 (chunks 2,3 / xhi / masks) is gated on the small weights
        # transfer via dummy copies that read it (the WAW dep delays each
        # dma issue), so the early wire belongs to the exp-chain-critical
        # data. DMA engines round-robin fairly across in-flight transfers.
        cstf8_sb = const.tile([128, 3, NDC, 2, DK], fp8, tag="cstf8")
        nc.sync.dma_start(out=cstf8_sb, in_=cstf8)
        xp01 = x8_pool.tile([128, 2, NDC, 2, KCW], fp8, tag="xp", name="xp01")
        xp23 = x8_pool.tile([128, 2, NDC, 2, KCW], fp8, tag="xp", name="xp23")
        x8s = [xp01[:, 0], xp01[:, 1], xp23[:, 0], xp23[:, 1]]
        PW = 2 * NDC * 2 * KCW
        nc.sync.dma_start(out=xp01, in_=x8p[:, 0:PW])
        cst32_sb = const.tile([128, 3 + DK], f32, tag="cst32")
        nc.sync.dma_start(out=cst32_sb, in_=cst32)

        # wave 2: xp23 gated on the cstf8 transfer (~10.5us, keeps the wire
        # warm); xhi/masks gated on xp01 itself (~12.7us) so chunks 0,1
        # never share the early wire with more than one other stream.
        xhi_sb = const.tile([128, NCHUNK, 256], bf16, tag="xhi")
        cst16_sb = const.tile([128, NCHUNK * DK + 256], bf16, tag="cst16")
        nc.gpsimd.tensor_copy(xp23[:, 0, 0, 0, 0:2], cstf8_sb[:, 0, 0, 0, 0:2])
        nc.sync.dma_start(out=xp23, in_=x8p[:, PW : 2 * PW])
        nc.vector.tensor_copy(xhi_sb[:, 0, 0:2], xp01[:, 0, 0, 0, 0:2])
        nc.sync.dma_start(out=xhi_sb, in_=xhip)
        nc.vector.tensor_copy(cst16_sb[:, 0:2], xp01[:, 0, 0, 0, 0:2])
        nc.sync.dma_start(out=cst16_sb, in_=cst16)

        wk8_sb = cstf8_sb[:, 0]
        wq8_sb = cstf8_sb[:, 1]
        wv8_sb = cstf8_sb[:, 2]
        bq_sb = cst32_sb[:, 0:1]
        bks_sb = cst32_sb[:, 1:2]
        bvv_sb = cst32_sb[:, 3 : 3 + DK]
        wv16_sb = cst16_sb[:, 0 : NCHUNK * DK]
        mask_sb = cst16_sb[:, NCHUNK * DK :]

        # ---- PE warmup: continuous PE activity from t0 until x8 chunks 0,1
        # land (pstate reaches full clock only after ~3us of uninterrupted
        # work) + pulls the exp ACT_TABLE_LOAD early.
        with tc.tile_pool(name="warmps", bufs=1, space="PSUM") as warmps:
            wsrc = sbufs.tile([128, 512], bf16, tag="wsrc")
            nc.vector.memset(wsrc, 0.0)
            wdst = warmps.tile([128, 512], f32, tag="warm")
            for _ in range(WARMUP_MMS):
                nc.tensor.matmul(
                    wdst, lhsT=wsrc[:, 0:128], rhs=wsrc, start=True, stop=True
                )
            wexp = sbufs.tile([128, 1], f32, tag="wexp")
            nc.scalar.activation(out=wexp, in_=wsrc[:, 0:1], func=Act.Exp, scale=1.0)

        # ---- q/k projections, key-chunk-major (fp8 DoubleRow).
        # kT positions 0..7 = own-query columns (x8 chunks 0,1 = qT source),
        # positions 8..15 = chunks 2,3. Per-piece sbuf tiles (dep tracking
        # is whole-tile). ACT copies only kt0; DVE does the rest.
        kts = [sbufs.tile([128, 512], bf16, tag=f"kt{i}", name=f"kt{i}")
               for i in range(4)]
        qT_lo = sbufs.tile([128, 512], bf16, tag="qTl")     # slots 0..3
        qT_hi = sbufs.tile([128, 512], bf16, tag="qTh")     # slots 4..7

        spool = tc.alloc_tile_pool(name="spool", bufs=2, space="PSUM")
        kqpool = tc.alloc_tile_pool(name="kqpool", bufs=2, space="PSUM")

        def kq_mms(dst_ps, w_sb, t):
            for dc in range(NDC):
                nc.tensor.matmul(
                    dst_ps,
                    lhsT=w_sb[:, dc, :, :],
                    rhs=x8s[t][:, dc],
                    start=(dc == 0),
                    stop=(dc == NDC - 1),
                    perf_mode=DR,
                )

        def dve_copy(dst, src_ps, scale, bias):
            nc.vector.tensor_scalar(
                out=dst, in0=src_ps, scalar1=float(scale), scalar2=bias,
                op0=Alu.mult, op1=Alu.add,
            )

        kps = [None] * NKC
        qps = [None] * 2
        # chunk 0: kT piece 0 first (ACT copy), then qT_lo
        kps[0] = kqpool.tile([128, 512], f32, tag="kps", name="kps0")
        kq_mms(kps[0], wk8_sb, 0)
        qps[0] = kqpool.tile([128, 512], f32, tag="qps", name="qps0")
        kq_mms(qps[0], wq8_sb, 0)
        nc.scalar.activation(
            out=kts[0], in_=kps[0], func=Act.Identity, bias=bks_sb,
            scale=SCALE / WS,
        )
        dve_copy(qT_lo, qps[0], 1.0 / WS, bq_sb)
        # chunk 1: qT_hi first (unblocks every exp), then kT piece 1
        qps[1] = kqpool.tile([128, 512], f32, tag="qps", name="qps1")
        kq_mms(qps[1], wq8_sb, 1)
        kps[1] = kqpool.tile([128, 512], f32, tag="kps", name="kps1")
        kq_mms(kps[1], wk8_sb, 1)
        dve_copy(qT_hi, qps[1], 1.0 / WS, bq_sb)
        dve_copy(kts[1], kps[1], SCALE / WS, bks_sb)

        # ---- attention: S^T/exp, v pairs, PV bursts ----
        pt_pool = ctx.enter_context(tc.tile_pool(name="pt_pool", bufs=12))

        pts = [None] * NBLK
        v_augs = [None] * (NBLK // 2)
        o_pss = [None] * NSLOT

        def kpos(j):
            # column position of local key block j in the reordered x8/kT
            return (j - 1) // 2 if j % 2 == 1 else NSLOT + j // 2

        def st_mms(j, s_ps, doff0):
            """S^T matmuls for key block j into s_ps at column offset doff0
            (pieces split at the qT_lo/qT_hi boundary; caller keeps every
            piece within one 512-col psum bank)."""
            q0 = 128 * (j // 2)
            kp = kpos(j)
            kt = kts[kp // 4]
            kp = kp % 4
            if q0 < 512:
                pieces = [(qT_lo, q0, 512 - q0), (qT_hi, 0, 512)]
            else:
                pieces = [(qT_hi, q0 - 512, 1024 - q0)]
            off = doff0
            for qtile, qoff, sz in pieces:
                nc.tensor.matmul(
                    s_ps[:, off : off + sz],
                    lhsT=kt[:, 128 * kp : 128 * kp + 128],
                    rhs=qtile[:, qoff : qoff + sz],
                    start=True,
                    stop=True,
                )
                off += sz

        def emit_mask(j, pt, off):
            # mask the frontier slot multiplicatively (exp(s+m) = exp(s)*m01):
            # even j -> maskA (wedge-dependent), odd j -> maskB (causal tri).
            # On gpsimd: DVE is near-saturated with psum copies/epilogues.
            sel = j % 2
            nc.gpsimd.tensor_mul(
                pt[:, off : off + 128],
                pt[:, off : off + 128],
                mask_sb[:, 128 * sel : 128 * (sel + 1)],
            )

        def emit_st(j):
            """S^T for key block j, one exp, frontier mask."""
            qn = NSLOT * 128 - 128 * (j // 2)
            pt = pt_pool.tile([128, qn], bf16, tag="pt", name=f"pt{j}")
            pts[j] = pt
            s_ps = spool.tile([128, 1024], f32, tag="st", name=f"s{j}")
            st_mms(j, s_ps, 128 * (j // 2))
            nc.scalar.activation(
                out=pt, in_=s_ps[:, 128 * (j // 2) : 1024], func=Act.Exp,
                scale=1.0,
            )
            emit_mask(j, pt, 0)

        def emit_st_pair(jA, jB, offB):
            """Two narrow S^T blocks sharing one psum tile and ONE exp (cuts
            ACT overhead and spool churn). offB is jB's column offset in the
            tile, chosen so both blocks stay within a psum bank (a gap
            between blocks is junk that the exp processes and nothing
            reads)."""
            qnA = NSLOT * 128 - 128 * (jA // 2)
            qnB = NSLOT * 128 - 128 * (jB // 2)
            tot = offB + qnB
            pt = pt_pool.tile([128, tot], bf16, tag="pt", name=f"pt{jA}_{jB}")
            pts[jA] = pt[:, 0:qnA]
            pts[jB] = pt[:, offB : offB + qnB]
            s_ps = spool.tile([128, 1024], f32, tag="st", name=f"s{jA}_{jB}")
            st_mms(jA, s_ps, 0)
            st_mms(jB, s_ps, offB)
            nc.scalar.activation(
                out=pt, in_=s_ps[:, 0:tot], func=Act.Exp, scale=1.0,
            )
            emit_mask(jA, pt, 0)
            emit_mask(jB, pt, offB)

        def emit_vpair(k):
            """v for key blocks 2k, 2k+1 in [key, dk] orientation. Blocks
            0,1 (pair 0) in bf16 from xhi (global key block 0 feeds the
            barely-averaged first rows of slot 0; wv16 is WS-prescaled so
            the uniform 1/WS copy applies); the rest via fp8 DoubleRow from
            the resident x8 chunks. No bias (folded into the epilogue)."""
            v_aug = const.tile([128, 2, DK + 1], bf16, tag=f"vaug{k}",
                               name=f"vaug{k}")
            v_augs[k] = v_aug
            nc.vector.memset(v_aug[:, :, DK : DK + 1], 1.0)
            vg_ps = vpool.tile([128, 2, DK], f32, tag="vps", name=f"vg{k}")
            for b in range(2):
                j = 2 * k + b
                if j < 2:
                    for c in range(NCHUNK):
                        nc.tensor.matmul(
                            vg_ps[:, b, :],
                            lhsT=xhi_sb[:, c, 128 * j : 128 * (j + 1)],
                            rhs=wv16_sb[:, 128 * c : 128 * (c + 1)],
                            start=(c == 0),
                            stop=(c == NCHUNK - 1),
                        )
                    continue
                kp = kpos(j)
                t, cb = kp // 4, kp % 4
                for dc in range(NDC):
                    nc.tensor.matmul(
                        vg_ps[:, b, :],
                        lhsT=x8s[t][:, dc, :, 128 * cb : 128 * (cb + 1)],
                        rhs=wv8_sb[:, dc],
                        start=(dc == 0),
                        stop=(dc == NDC - 1),
                        perf_mode=DR,
                    )
            nc.vector.tensor_scalar(
                out=v_aug[:, :, 0:DK], in0=vg_ps, scalar1=float(1.0 / WS),
                scalar2=0.0, op0=Alu.mult, op1=Alu.add,
            )

        def emit_pv(p, j_lo, j_hi):
            if j_lo == 0:
                o_pss[p] = opool.tile([128, DK + 1], f32, tag="o",
                                      name=f"o_ps{p}")
            o_ps = o_pss[p]
            for jj in range(j_lo, j_hi):
                nc.tensor.matmul(
                    o_ps,
                    lhsT=pts[jj][:, 128 * (p - jj // 2) : 128 * (p - jj // 2) + 128],
                    rhs=v_augs[jj // 2][:, jj % 2, :],
                    start=(jj == 0),
                    stop=(jj == 2 * p + 1),
                )

        def emit_finish(p):
            """out = o_ps * (1/den) + bv, then DMA out."""
            o_ps = o_pss[p]
            rcp = out_pool.tile([128, 1], f32, tag="rcp")
            nc.vector.reciprocal(rcp, o_ps[:, DK : DK + 1])
            ob = out_pool.tile([128, DK], f32, tag="ob")
            nc.vector.scalar_tensor_tensor(
                out=ob, in0=o_ps[:, 0:DK], scalar=rcp, in1=bvv_sb,
                op0=Alu.mult, op1=Alu.add,
            )
            nc.sync.dma_start(out=o[128 * p : 128 * (p + 1), :], in_=ob)

        def emit_burst(p):
            emit_pv(p, 0, 2 * p + 2)
            emit_finish(p)

        # ACT chain order (burst p needs the consecutive exp prefix
        # 0..2p+1): 1,0,3,2,5,4,7,6,P(8,9),P(10,11),P(12,13),P(14,15).
        # PE queue: each S^T sits as close to the head as possible when its
        # spool slot frees; v pairs and bursts are ready-when-reached
        # filler between S^Ts; bursts 6,7 split so only blocks 12+ trail
        # the last exp.
        emit_st(1)
        # chunks 2,3 arrive with wave 2; kT pieces 2,3 (psum slots recycled)
        for t in (2, 3):
            kps[t] = kqpool.tile([128, 512], f32, tag="kps", name=f"kps{t}")
            kq_mms(kps[t], wk8_sb, t)
            dve_copy(kts[t], kps[t], SCALE / WS, bks_sb)
        kqpool.release()
        vpool = tc.alloc_tile_pool(name="vpool", bufs=1, space="PSUM")
        opool = tc.alloc_tile_pool(name="opool", bufs=3, space="PSUM")

        emit_st(0); emit_st(3)
        emit_vpair(1)
        emit_st(2)
        emit_vpair(0)
        emit_burst(0)
        emit_vpair(2)
        emit_st(5)
        emit_burst(1)
        emit_st(4)
        emit_vpair(3)
        emit_st(7)
        emit_vpair(4)
        emit_burst(2)
        emit_st(6)
        emit_vpair(5)
        emit_st_pair(8, 9, 512)
        emit_burst(3)
        emit_vpair(6)
        emit_st_pair(10, 11, 512)   # j10 [0:384], j11 [512:896]
        emit_vpair(7)
        emit_burst(4)
        emit_st_pair(12, 13, 256)
        emit_burst(5)
        emit_st_pair(14, 15, 128)
        emit_pv(6, 0, 12)
        emit_pv(7, 0, 12)
        emit_pv(6, 12, 14)
        emit_finish(6)
        emit_pv(7, 12, 16)
        emit_finish(7)

        opool.release()
        vpool.release()
        spool.release()

    nc.compile()
    return nc


def get_built():
    global _built
    if _built is None:
        _built = _build()
    return _built


def _pos2glob(h):
    if h == 0:
        return list(range(NBLK))
    return [j + 1 if j % 2 == 0 else j - 1 for j in range(NBLK)]


def _pack_w_fp8(W):
    """[D, DK] -> [128, NDC*2*DK] e4m3: [p, ((dc*2+i)*DK)+d] = e4m3(WS*W[256dc+128i+p, d])."""
    import ml_dtypes
    Ws = np.asarray(W, np.float32) * WS
    return np.ascontiguousarray(
        Ws.reshape(NDC, 2, 128, DK).transpose(2, 0, 1, 3)
        .reshape(128, NDC * 2 * DK).astype(ml_dtypes.float8_e4m3)
    )


def _pack_w_bf16(W):
    """[D, DK] -> [128, NCHUNK*DK] bf16 (xWS prescale), col block c = rows 128c..."""
    import ml_dtypes
    return np.ascontiguousarray(
        (np.asarray(W, np.float32) * WS).reshape(NCHUNK, 128, DK)
        .transpose(1, 0, 2).reshape(128, NCHUNK * DK).astype(ml_dtypes.bfloat16)
    )


def make_in_map(x_b, Wq, bq, Wk, bk, Wv, bv, h, xT_pre=None, x8T_pre=None):
    """Build one core's input dict. x_b: [T, D] fp32 for this core's batch.
    x8T_pre: optional precomputed transposed fp8 copy (shared by both wedge
    cores of a batch; h=0 uses as-is, h=1 column-permutes). xT_pre unused
    (kept for signature compat)."""
    import ml_dtypes
    bf = ml_dtypes.bfloat16
    if x8T_pre is None:
        x8T_pre = np.ascontiguousarray(x_b.T.astype(ml_dtypes.float8_e4m3))
    if h == 0:
        x8T_loc = x8T_pre
    else:
        p2g = _pos2glob(h)
        cols = np.concatenate([np.arange(128 * g, 128 * (g + 1)) for g in p2g])
        x8T_loc = np.ascontiguousarray(x8T_pre[:, cols])
    # x8 column order: own-query blocks (odd locals, slot order) first, then
    # the even locals -- chunks 0,1 feed the q projection and kT pos 0..7
    korder = list(range(1, NBLK, 2)) + list(range(0, NBLK, 2))
    qcols = np.concatenate([np.arange(128 * j, 128 * (j + 1)) for j in korder])
    # x8p[p, ((t*NDC+dc)*2+i)*KCW + c] = x8T[256dc+128i+p, qcols[512t+c]]
    x8q = x8T_loc[:, qcols]                       # [1024 rows d, 2048 cols]
    x8p = np.ascontiguousarray(
        x8q.reshape(NDC, 2, 128, NKC, KCW).transpose(2, 3, 0, 1, 4)
        .reshape(128, NKC * NDC * 2 * KCW)
    )
    maskA = (np.ones if h == 0 else np.zeros)((128, 128), bf)
    kk = np.arange(128)
    maskB = np.where(kk[:, None] <= kk[None, :], 1.0, 0.0).astype(bf)
    # cst32: [bq, bk*SCALE, pad, bvv(128 cols, bv broadcast to all parts)]
    cst32 = np.zeros((128, 3 + DK), np.float32)
    cst32[:, 0] = np.asarray(bq, np.float32)
    cst32[:, 1] = np.asarray(bk, np.float32) * SCALE
    cst32[:, 3:] = np.asarray(bv, np.float32)[None, :]
    cstf8 = np.concatenate(
        [_pack_w_fp8(Wk), _pack_w_fp8(Wq), _pack_w_fp8(Wv)], axis=1
    )
    cst16 = np.concatenate([_pack_w_bf16(Wv), maskA, maskB], axis=1)
    # xhip: bf16 x^T for LOCAL key blocks 0,1 (h=0: global cols 0:256;
    # h=1: global 128:256 then 0:128), layout [p, c*256 + col]
    loc01 = (np.r_[0:256] if h == 0
             else np.r_[128:256, 0:128])
    xhi = x_b[loc01, :].T.astype(bf)              # [1024 d, 256]
    xhip = np.ascontiguousarray(
        xhi.reshape(NCHUNK, 128, 256).transpose(1, 0, 2)
        .reshape(128, NCHUNK * 256)
    )
    return {
        "x8p": x8p,
        "xhip": xhip,
        "cstf8": np.ascontiguousarray(cstf8),
        "cst32": np.ascontiguousarray(cst32),
        "cst16": np.ascontiguousarray(cst16),
    }


def gather_out(results):
    """results: list of 8 dicts with 'o' [1024, 128] -> full [B, T, DK]."""
    out = np.zeros((B, T, DK), np.float32)
    for core in range(8):
        b, h = core // 2, core % 2
        ob = results[core]["o"]
        for p in range(NSLOT):
            g = 2 * p + 1 - h
            out[b, 128 * g : 128 * (g + 1), :] = ob[128 * p : 128 * (p + 1), :]
    return out


def kernel(x, Wq, bq, Wk, bk, Wv, bv):
    import ml_dtypes
    from concourse.bass_utils import run_bass_kernel_spmd

    x = np.asarray(x, np.float32)
    args = [np.asarray(a, np.float32) for a in (Wq, bq, Wk, bk, Wv, bv)]
    nc = get_built()
    # one transpose+cast per batch, shared by its two wedge cores
    x8T_pres = [np.ascontiguousarray(x[b].T.astype(ml_dtypes.float8_e4m3))
                for b in range(B)]
    in_maps = [
        make_in_map(x[core // 2], args[0], args[1], args[2], args[3], args[4],
                    args[5], core % 2, x8T_pre=x8T_pres[core // 2])
        for core in range(8)
    ]
    res = run_bass_kernel_spmd(nc, in_maps, core_ids=list(range(8)))
    return gather_out(res.results)


if __name__ == "__main__":
    rng = np.random.default_rng(0)
    x = rng.standard_normal((B, T, D), dtype=np.float32)
    Wq = rng.standard_normal((D, DK), dtype=np.float32) * 0.03
    out = kernel(x, Wq, np.zeros(DK, np.float32), Wq, np.zeros(DK, np.float32),
                 Wq, np.zeros(DK, np.float32))
    print(out.shape)
